# revision 50
# baseline (speedup 1.0000x reference)
"""AttentiveMatch kernel for Trainium2 (8 NeuronCores, data-parallel over batch).

Reference math (per batch):
    pn = l2norm(p); qn = l2norm(q)
    w  = -(pn @ qn^T) / D          # [S,S]
    mv = (w @ q) / S               # [S,D]
    mn = l2norm(mv)
    out = -mean(pn * mn, -1)       # [S]

Signs/scalars fold away: out_i = (1/D) * (p_i . M_i) / (|p_i| |M_i|)
with M_i = sum_j (G_ji / |q_j|) q_j and G = q p^T.

fp8 pipeline with the row-norm folded into q on the host:
    qs = fp8(sqrt(1/|q8_j|) * q)   shipped in natural + transposed layouts
    b  = qs @ p8^T                 [S,S]  mm1, fp8 DoubleRow -> = sqrt(rq)*G
    b8 = fp8(b)                    PSUM->SBUF copy (DVE)
    h  = b^2                       bf16, ACT Square from PSUM / DVE from b8
    M^T = qs^T b8                  [D,S]  mm2, fp8 DoubleRow
    dot_i = sum_j h[j,i]           bf16 ones-weight matmul row
    ss_i  = sum_d (M^T)^2[d,i]     bf16 Square + ones matmul row
    out_i = dot_i / (D |p8_i| sqrt(ss_i))

Finals run row-wise straight off the PSUM rows (ACT sqrt, DVE fast
reciprocal + mul, Pool mul); each batch DMAs its [1,S] result out.

DoubleRow notes (measured on HW): dual-fp8 streams at ~216ns per
[256K x 128M x 512N] instruction -- 2x bf16 MACs/s; weight loads are
256 rows and do not fully hide, and accumulation-group starts cost
~+145ns. Dual LdWeights requires contiguous [128,2,M] weight pairs
with M a multiple of 16; dual matmul outputs must start at partition 0.
"""

import os
import sys

for _p in ("/opt/trn_rl_repo",):
    if _p not in sys.path:
        sys.path.append(_p)

import numpy as np
import ml_dtypes

import concourse.bacc as bacc
import concourse.mybir as mybir
import concourse.tile as tile
from concourse.bass_utils import run_bass_kernel_spmd

B, S, D = 64, 512, 768
NCORES = 8
BP = B // NCORES          # batches per core
ST = S // 128             # s tiles (4)
KT = D // 128             # d tiles (6)
F32 = mybir.dt.float32
BF16 = mybir.dt.bfloat16
F8 = mybir.dt.float8e4
AF = mybir.ActivationFunctionType
ALU = mybir.AluOpType
DR = mybir.MatmulPerfMode.DoubleRow
NPF8 = ml_dtypes.float8_e4m3

_NC = None

if os.environ.get("KERNEL_LDW_OPT", "0") == "1":
    # pipeline LdWeights with the previous matmul's stream
    import concourse.bass_utils as _bu

    _orig_run_command = _bu.run_command

    def _patched_run_command(cmd, **kw):
        cmd = [
            ("--enable-ldw-opt=true" if c == "--enable-ldw-opt=false" else c)
            for c in cmd
        ]
        return _orig_run_command(cmd, **kw)

    _bu.run_command = _patched_run_command


def _build():
    nc = bacc.Bacc("TRN2", target_bir_lowering=False, debug=False, num_devices=NCORES)
    # weight layouts keep each [128, 2, 128] DoubleRow pair contiguous
    qst_d = nc.dram_tensor("qst", [BP, 128, KT // 2, ST, 2, 128], F8,
                           kind="ExternalInput")
    pt_d = nc.dram_tensor("pt", [BP, 128, KT, S], F8, kind="ExternalInput")
    qs_d = nc.dram_tensor("qs", [BP, 128, ST // 2, KT, 2, 128], F8,
                          kind="ExternalInput")
    rp_d = nc.dram_tensor("rp", [1, BP * S], F32, kind="ExternalInput")
    out_d = nc.dram_tensor("out", [BP, S], F32, kind="ExternalOutput")

    with tile.TileContext(nc) as tc:
        with (
            tc.tile_pool(name="cst", bufs=1) as cst,
            tc.tile_pool(name="inp", bufs=3) as inp,
            tc.tile_pool(name="bsb", bufs=2) as bsb,
            tc.tile_pool(name="s2b", bufs=2) as s2b,
            tc.tile_pool(name="gps", bufs=4, space="PSUM") as gps,
            tc.tile_pool(name="mps", bufs=3, space="PSUM") as mps,
            tc.tile_pool(name="rps", bufs=1, space="PSUM") as rps,
            tc.tile_pool(name="res", bufs=1) as res,
        ):
            ones16 = cst.tile([128, 1], BF16)
            nc.gpsimd.memset(ones16[:], 1.0)
            # dual-fp8 ones weights, 64 replicated cols (dual matmul output
            # must start at partition 0); ss result is read from partition 32
            ones8 = cst.tile([128, 2, 64], F8)
            nc.gpsimd.memset(ones8[:], 1.0)

            rpt = res.tile([1, BP * S], F32)
            # finals tiles are reused across batches (off critical path)
            sd = res.tile([1, S], F32)
            rs = res.tile([1, S], F32)
            w1 = res.tile([1, S], F32)
            ow = res.tile([1, S], F32)

            # per-batch state carried across the software pipeline
            st_rows = [None] * BP
            st_s2 = [None] * BP
            st_h = [None] * BP

            def load(b, split):
                nch = 3 if split else 1
                w = KT // 2 // nch   # k-pairs per chunk
                qc, pc = [], []
                # batch 0: split the first chunks in halves across all three
                # rings so mm1's first instruction starts ~2x earlier
                qrings = [nc.sync, nc.scalar, nc.gpsimd]
                prings = [nc.scalar, nc.sync, nc.gpsimd]
                for c in range(nch):
                    t = inp.tile([128, w, ST, 2, 128], F8, tag=f"qst{c}_{nch}")
                    if split and c == 0:
                        nc.sync.dma_start(t[:, :, 0:2], qst_d[b, :, 0:w, 0:2])
                        nc.gpsimd.dma_start(t[:, :, 2:4],
                                            qst_d[b, :, 0:w, 2:4])
                    else:
                        (qrings[c] if split else nc.sync).dma_start(
                            t[:], qst_d[b, :, c * w:(c + 1) * w])
                    qc.append(t)
                    t = inp.tile([128, 2 * w, S], F8, tag=f"pt{c}_{nch}")
                    if split and c == 0:
                        nc.scalar.dma_start(t[:, 0:1, :], pt_d[b, :, 0:1, :])
                        nc.gpsimd.dma_start(t[:, 1:2, :], pt_d[b, :, 1:2, :])
                    else:
                        (prings[c] if split else nc.gpsimd).dma_start(
                            t[:], pt_d[b, :, 2 * c * w:2 * (c + 1) * w, :])
                    pc.append(t)
                qn = inp.tile([128, ST // 2, KT, 2, 128], F8, tag="qs")
                # batch 0: gpsimd ring already carries two chunks; use sync
                (nc.sync if split else nc.gpsimd).dma_start(qn[:], qs_d[b])
                return qc, pc, qn

            def ss_rows(b):
                # 3 dual ones-matmuls over fp8 s2: rows[32] = ss/256
                rows = st_rows[b]
                s2 = st_s2[b]
                for c in range(KT // 2):
                    nc.tensor.matmul(
                        rows[0:64, :], lhsT=ones8[:],
                        rhs=s2[:, 2 * c:2 * c + 2, :],
                        start=(c == 0), stop=(c == KT // 2 - 1),
                        perf_mode=DR,
                    )

            def finish(b):
                # row-wise finals straight off the PSUM rows:
                # out = dot * rp / (D * sqrt(ss)).  The last batch's chain is
                # the kernel tail, so run it in half-rows to pipeline
                # ACT / DVE / DMA instead of serializing the full row.
                rows = st_rows[b]
                nh = 2 if b == BP - 1 else 1
                hw_ = S // nh
                for i in range(nh):
                    sl = slice(i * hw_, (i + 1) * hw_)
                    nc.scalar.activation(sd[:, sl], rows[32:33, sl], AF.Sqrt,
                                         scale=(16.0 * D) * (16.0 * D))
                    nc.vector.reciprocal_approx_fast(rs[:, sl], sd[:, sl])
                    nc.vector.tensor_mul(w1[:, sl], rows[64:65, sl],
                                         rpt[:, b * S + i * hw_:
                                             b * S + (i + 1) * hw_])
                    nc.vector.tensor_mul(ow[:, sl], w1[:, sl], rs[:, sl])
                    nc.gpsimd.dma_start(out_d[b:b + 1, sl], ow[:, sl])

            loads = load(0, True)
            nc.sync.dma_start(rpt[:], rp_d[:])
            for b in range(BP):
                qc, pc, qn = loads
                kw = (KT // 2) // len(qc)  # k-pairs per chunk

                # mm1: b_pre[j,i] = sum_d qs[j,d] p8[i,d], DoubleRow k-pairs
                bp = bsb.tile([128, ST, S], F8, tag="bp")
                h = s2b.tile([128, ST, S], BF16, tag="h")
                st_h[b] = h
                for jt in range(ST):
                    g = gps.tile([128, S], F32, tag="g")
                    for c in range(KT // 2):
                        kc, ko = divmod(c, kw)
                        nc.tensor.matmul(
                            g[:],
                            lhsT=qc[kc][:, ko, jt],
                            rhs=pc[kc][:, 2 * ko:2 * ko + 2, :],
                            start=(c == 0), stop=(c == KT // 2 - 1),
                            perf_mode=DR,
                        )
                    nc.vector.tensor_copy(bp[:, jt, :], g[:])
                    # h = b^2 in bf16, split across ACT (from PSUM) and DVE
                    if jt < 2:
                        nc.scalar.activation(h[:, jt, :], g[:], AF.Square)
                    else:
                        nc.vector.scalar_tensor_tensor(
                            h[:, jt, :], bp[:, jt, :], 1.0, bp[:, jt, :],
                            ALU.mult, ALU.mult)

                # software pipeline: prev batch's ss reduction on the PE
                # here, after its s2 tiles have certainly landed
                if b > 0:
                    ss_rows(b - 1)

                # prefetch next batch while mm2 runs
                if b + 1 < BP:
                    loads = load(b + 1, False)

                rows = rps.tile([65, S], F32, tag="rows")
                st_rows[b] = rows

                # mm2: mt[k] = sum_j qs[j,d] b8[j,i] (fp8 DoubleRow jt-pairs)
                s2 = s2b.tile([128, KT, S], F8, tag="s2")
                st_s2[b] = s2
                for k in range(KT):
                    mt = mps.tile([128, S], F32, tag="mt")
                    for jp in range(ST // 2):
                        nc.tensor.matmul(
                            mt[:],
                            lhsT=qn[:, jp, k],
                            rhs=bp[:, 2 * jp:2 * jp + 2, :],
                            start=(jp == 0), stop=(jp == ST // 2 - 1),
                            perf_mode=DR,
                        )
                    # s2 = (mt/16)^2 in fp8 (single PSUM read); quantizing
                    # after the square keeps the ss error ~0.2%
                    nc.scalar.activation(s2[:, k, :], mt[:], AF.Square,
                                         scale=1.0 / 16.0)

                # dot row at partition 64 (ss writes 0:64 later)
                for jt in range(ST):
                    nc.tensor.matmul(
                        rows[64:65, :], lhsT=ones16[:], rhs=h[:, jt, :],
                        start=(jt == 0), stop=(jt == ST - 1),
                    )

                if b > 0:
                    finish(b - 1)

            ss_rows(BP - 1)
            finish(BP - 1)
    nc.compile()
    return nc


def _get_nc():
    global _NC
    if _NC is None:
        _NC = _build()
    return _NC


def _prep_inputs(p, q):
    p = np.asarray(p, dtype=np.float32)
    q = np.asarray(q, dtype=np.float32)
    p8 = p.astype(NPF8)
    p8f = p8.astype(np.float32)
    q8f = q.astype(NPF8).astype(np.float32)
    rq = 1.0 / np.sqrt((q8f * q8f).sum(-1))            # [B,S]
    rp = (1.0 / np.sqrt((p8f * p8f).sum(-1))).astype(np.float32)
    qs8 = (np.sqrt(rq)[..., None] * q).astype(NPF8)    # [B,S,D] fp8

    # mm1 weights: [core, b, dpart, kp, jt, e, jc] with d = (2kp+e)*128+dpart,
    # j = jt*128 + jc  (each [128, 2, 128] DoubleRow pair contiguous)
    qst = np.ascontiguousarray(
        qs8.reshape(NCORES, BP, ST, 128, KT // 2, 2, 128)
        .transpose(0, 1, 6, 4, 2, 5, 3)
    )
    # mm1 moving: [core, b, part, k, i] with d = k*128 + part
    pt = np.ascontiguousarray(
        p8.reshape(NCORES, BP, S, KT, 128).transpose(0, 1, 4, 3, 2)
    )
    # mm2 weights: [core, b, jpart, jp, k, e, dc] with j = (2jp+e)*128+jpart,
    # d = k*128 + dc
    qsn = np.ascontiguousarray(
        qs8.reshape(NCORES, BP, ST // 2, 2, 128, KT, 128)
        .transpose(0, 1, 4, 2, 5, 3, 6)
    )
    rpc = np.ascontiguousarray(rp.reshape(NCORES, 1, BP * S))
    return [
        {"qst": qst[c], "pt": pt[c], "qs": qsn[c], "rp": rpc[c]}
        for c in range(NCORES)
    ]


def _postprocess(results):
    o = np.stack([np.asarray(r["out"], dtype=np.float32) for r in results])
    return np.ascontiguousarray(o.reshape(B, 1, S))


def _run(inputs, trace=False, **kw):
    nc = _get_nc()
    in_maps = _prep_inputs(inputs["p"], inputs["q"])
    res = run_bass_kernel_spmd(nc, in_maps, list(range(NCORES)), trace=trace, **kw)
    return _postprocess(res.results), res


def kernel(p, q):
    out, _ = _run({"p": p, "q": q})
    return out


# revision 51
# speedup vs baseline: 1.1382x; 1.1382x over previous
"""AttentiveMatch kernel for Trainium2 (8 NeuronCores, data-parallel over batch).

Reference math (per batch):
    pn = l2norm(p); qn = l2norm(q)
    w  = -(pn @ qn^T) / D          # [S,S]
    mv = (w @ q) / S               # [S,D]
    mn = l2norm(mv)
    out = -mean(pn * mn, -1)       # [S]

Signs/scalars fold away: out_i = (1/D) * (p_i . M_i) / (|p_i| |M_i|)
with M_i = sum_j (G_ji / |q_j|) q_j and G = q p^T.

fp8 pipeline with the row-norm folded into q on the host:
    qs = fp8(sqrt(1/|q8_j|) * q)   shipped in natural + transposed layouts
    b  = qs @ p8^T                 [S,S]  mm1, fp8 DoubleRow -> = sqrt(rq)*G
    b8 = fp8(b)                    PSUM->SBUF copy (DVE)
    h  = b^2                       bf16, ACT Square from PSUM / DVE from b8
    M^T = qs^T b8                  [D,S]  mm2, fp8 DoubleRow
    dot_i = sum_j h[j,i]           bf16 ones-weight matmul row
    ss_i  = sum_d (M^T)^2[d,i]     bf16 Square + ones matmul row
    out_i = dot_i / (D |p8_i| sqrt(ss_i))

Finals run row-wise straight off the PSUM rows (ACT sqrt, DVE fast
reciprocal + mul, Pool mul); each batch DMAs its [1,S] result out.

DoubleRow notes (measured on HW): dual-fp8 streams at ~216ns per
[256K x 128M x 512N] instruction -- 2x bf16 MACs/s; weight loads are
256 rows and do not fully hide, and accumulation-group starts cost
~+145ns. Dual LdWeights requires contiguous [128,2,M] weight pairs
with M a multiple of 16; dual matmul outputs must start at partition 0.
"""

import os
import sys

for _p in ("/opt/trn_rl_repo",):
    if _p not in sys.path:
        sys.path.append(_p)

import numpy as np
import ml_dtypes

import concourse.bacc as bacc
import concourse.mybir as mybir
import concourse.tile as tile
from concourse.bass_utils import run_bass_kernel_spmd

B, S, D = 64, 512, 768
NCORES = 8
BP = B // NCORES          # batches per core
ST = S // 128             # s tiles (4)
KT = D // 128             # d tiles (6)
F32 = mybir.dt.float32
BF16 = mybir.dt.bfloat16
F8 = mybir.dt.float8e4
AF = mybir.ActivationFunctionType
ALU = mybir.AluOpType
DR = mybir.MatmulPerfMode.DoubleRow
NPF8 = ml_dtypes.float8_e4m3

_NC = None

if os.environ.get("KERNEL_LDW_OPT", "0") == "1":
    # pipeline LdWeights with the previous matmul's stream
    import concourse.bass_utils as _bu

    _orig_run_command = _bu.run_command

    def _patched_run_command(cmd, **kw):
        cmd = [
            ("--enable-ldw-opt=true" if c == "--enable-ldw-opt=false" else c)
            for c in cmd
        ]
        return _orig_run_command(cmd, **kw)

    _bu.run_command = _patched_run_command


def _build():
    nc = bacc.Bacc("TRN2", target_bir_lowering=False, debug=False, num_devices=NCORES)
    # weight layouts keep each [128, 2, 128] DoubleRow pair contiguous
    qst_d = nc.dram_tensor("qst", [BP, 128, KT // 2, ST, 2, 128], F8,
                           kind="ExternalInput")
    pt_d = nc.dram_tensor("pt", [BP, 128, KT, S], F8, kind="ExternalInput")
    qs_d = nc.dram_tensor("qs", [BP, 128, ST // 2, KT, 2, 128], F8,
                          kind="ExternalInput")
    rp_d = nc.dram_tensor("rp", [1, BP * S], F32, kind="ExternalInput")
    out_d = nc.dram_tensor("out", [BP, S], F32, kind="ExternalOutput")

    with tile.TileContext(nc) as tc:
        with (
            tc.tile_pool(name="cst", bufs=1) as cst,
            tc.tile_pool(name="inp", bufs=3) as inp,
            tc.tile_pool(name="bsb", bufs=2) as bsb,
            tc.tile_pool(name="s2b", bufs=2) as s2b,
            tc.tile_pool(name="st", bufs=2) as st,
            tc.tile_pool(name="gps", bufs=3, space="PSUM") as gps,
            tc.tile_pool(name="mps", bufs=3, space="PSUM") as mps,
            tc.tile_pool(name="rps", bufs=2, space="PSUM") as rps,
            tc.tile_pool(name="res", bufs=1) as res,
        ):
            ones16 = cst.tile([128, 1], BF16)
            nc.gpsimd.memset(ones16[:], 1.0)
            # dual-fp8 ones weights, 64 replicated cols (dual matmul output
            # must start at partition 0); ss result is read from partition 32
            ones8 = cst.tile([128, 2, 64], F8)
            nc.gpsimd.memset(ones8[:], 1.0)

            rpt = res.tile([1, BP * S], F32)
            # finals tiles are reused across batches (off critical path)
            sd = res.tile([1, S], F32)
            rs = res.tile([1, S], F32)
            w1 = res.tile([1, S], F32)
            ow = res.tile([1, S], F32)

            # per-batch state carried across the software pipeline
            st_rows = [None] * BP
            st_s2 = [None] * BP
            st_h = [None] * BP

            def load(b, split):
                nch = 3 if split else 1
                w = KT // 2 // nch   # k-pairs per chunk
                qc, pc = [], []
                # batch 0: split the first chunks in halves across all three
                # rings so mm1's first instruction starts ~2x earlier
                qrings = [nc.sync, nc.scalar, nc.gpsimd]
                prings = [nc.scalar, nc.sync, nc.gpsimd]
                for c in range(nch):
                    t = inp.tile([128, w, ST, 2, 128], F8, tag=f"qst{c}_{nch}")
                    if split and c == 0:
                        nc.sync.dma_start(t[:, :, 0:2], qst_d[b, :, 0:w, 0:2])
                        nc.gpsimd.dma_start(t[:, :, 2:4],
                                            qst_d[b, :, 0:w, 2:4])
                    else:
                        (qrings[c] if split else nc.sync).dma_start(
                            t[:], qst_d[b, :, c * w:(c + 1) * w])
                    qc.append(t)
                    t = inp.tile([128, 2 * w, S], F8, tag=f"pt{c}_{nch}")
                    if split and c == 0:
                        nc.scalar.dma_start(t[:, 0:1, :], pt_d[b, :, 0:1, :])
                        nc.gpsimd.dma_start(t[:, 1:2, :], pt_d[b, :, 1:2, :])
                    else:
                        (prings[c] if split else nc.gpsimd).dma_start(
                            t[:], pt_d[b, :, 2 * c * w:2 * (c + 1) * w, :])
                    pc.append(t)
                qn = inp.tile([128, ST // 2, KT, 2, 128], F8, tag="qs")
                # batch 0: gpsimd ring already carries two chunks; use sync
                (nc.sync if split else nc.gpsimd).dma_start(qn[:], qs_d[b])
                return qc, pc, qn

            def ss_rows(b):
                # 3 dual ones-matmuls over fp8 s2: rows[32] = ss/256
                rows = st_rows[b]
                s2 = st_s2[b]
                for c in range(KT // 2):
                    nc.tensor.matmul(
                        rows[0:64, :], lhsT=ones8[:],
                        rhs=s2[:, 2 * c:2 * c + 2, :],
                        start=(c == 0), stop=(c == KT // 2 - 1),
                        perf_mode=DR,
                    )

            def finish(b):
                # row-wise finals straight off the PSUM rows:
                # out = dot * rp / (D * sqrt(ss)).  The last batch's chain is
                # the kernel tail, so run it in half-rows to pipeline
                # ACT / DVE / DMA instead of serializing the full row.
                rows = st_rows[b]
                nh = 2 if b == BP - 1 else 1
                hw_ = S // nh
                for i in range(nh):
                    sl = slice(i * hw_, (i + 1) * hw_)
                    nc.scalar.activation(sd[:, sl], rows[32:33, sl], AF.Sqrt,
                                         scale=(16.0 * D) * (16.0 * D))
                    nc.vector.reciprocal_approx_fast(rs[:, sl], sd[:, sl])
                    nc.vector.tensor_mul(w1[:, sl], rows[64:65, sl],
                                         rpt[:, b * S + i * hw_:
                                             b * S + (i + 1) * hw_])
                    nc.vector.tensor_mul(ow[:, sl], w1[:, sl], rs[:, sl])
                    nc.gpsimd.dma_start(out_d[b:b + 1, sl], ow[:, sl])

            loads = load(0, True)
            nc.sync.dma_start(rpt[:], rp_d[:])
            for b in range(BP):
                qc, pc, qn = loads
                kw = (KT // 2) // len(qc)  # k-pairs per chunk

                # mm1: b_pre[j,i] = sum_d qs[j,d] p8[i,d], DoubleRow k-pairs
                bp = bsb.tile([128, ST, S], F8, tag="bp")
                h = s2b.tile([128, ST, S], BF16, tag="h")
                st_h[b] = h
                for jt in range(ST):
                    g = gps.tile([128, S], F32, tag="g")
                    for c in range(KT // 2):
                        kc, ko = divmod(c, kw)
                        nc.tensor.matmul(
                            g[:],
                            lhsT=qc[kc][:, ko, jt],
                            rhs=pc[kc][:, 2 * ko:2 * ko + 2, :],
                            start=(c == 0), stop=(c == KT // 2 - 1),
                            perf_mode=DR,
                        )
                    nc.vector.tensor_copy(bp[:, jt, :], g[:])
                    # h = b^2 in bf16, split across ACT (from PSUM) and DVE
                    if jt < 2:
                        nc.scalar.activation(h[:, jt, :], g[:], AF.Square)
                    else:
                        nc.vector.scalar_tensor_tensor(
                            h[:, jt, :], bp[:, jt, :], 1.0, bp[:, jt, :],
                            ALU.mult, ALU.mult)

                # software pipeline: prev batch's ss reduction on the PE
                # here, after its s2 tiles have certainly landed
                if b > 0:
                    ss_rows(b - 1)

                # prefetch next batch while mm2 runs
                if b + 1 < BP:
                    loads = load(b + 1, False)

                rows = rps.tile([65, S], F32, tag="rows")
                st_rows[b] = rows

                # mm2: mt[k] = sum_j qs[j,d] b8[j,i] (fp8 DoubleRow jt-pairs)
                s2 = s2b.tile([128, KT, S], F8, tag="s2")
                st_s2[b] = s2
                for k in range(KT):
                    mt = mps.tile([128, S], F32, tag="mt")
                    for jp in range(ST // 2):
                        nc.tensor.matmul(
                            mt[:],
                            lhsT=qn[:, jp, k],
                            rhs=bp[:, 2 * jp:2 * jp + 2, :],
                            start=(jp == 0), stop=(jp == ST // 2 - 1),
                            perf_mode=DR,
                        )
                    # s2 = (mt/16)^2 in fp8 (single PSUM read); quantizing
                    # after the square keeps the ss error ~0.2%
                    nc.scalar.activation(s2[:, k, :], mt[:], AF.Square,
                                         scale=1.0 / 16.0)

                # dot row at partition 64 (ss writes 0:64 later)
                for jt in range(ST):
                    nc.tensor.matmul(
                        rows[64:65, :], lhsT=ones16[:], rhs=h[:, jt, :],
                        start=(jt == 0), stop=(jt == ST - 1),
                    )

                if b > 0:
                    finish(b - 1)

            ss_rows(BP - 1)
            finish(BP - 1)
    nc.compile()
    return nc


def _get_nc():
    global _NC
    if _NC is None:
        _NC = _build()
    return _NC


def _prep_inputs(p, q):
    p = np.asarray(p, dtype=np.float32)
    q = np.asarray(q, dtype=np.float32)
    p8 = p.astype(NPF8)
    p8f = p8.astype(np.float32)
    q8f = q.astype(NPF8).astype(np.float32)
    rq = 1.0 / np.sqrt((q8f * q8f).sum(-1))            # [B,S]
    rp = (1.0 / np.sqrt((p8f * p8f).sum(-1))).astype(np.float32)
    qs8 = (np.sqrt(rq)[..., None] * q).astype(NPF8)    # [B,S,D] fp8

    # mm1 weights: [core, b, dpart, kp, jt, e, jc] with d = (2kp+e)*128+dpart,
    # j = jt*128 + jc  (each [128, 2, 128] DoubleRow pair contiguous)
    qst = np.ascontiguousarray(
        qs8.reshape(NCORES, BP, ST, 128, KT // 2, 2, 128)
        .transpose(0, 1, 6, 4, 2, 5, 3)
    )
    # mm1 moving: [core, b, part, k, i] with d = k*128 + part
    pt = np.ascontiguousarray(
        p8.reshape(NCORES, BP, S, KT, 128).transpose(0, 1, 4, 3, 2)
    )
    # mm2 weights: [core, b, jpart, jp, k, e, dc] with j = (2jp+e)*128+jpart,
    # d = k*128 + dc
    qsn = np.ascontiguousarray(
        qs8.reshape(NCORES, BP, ST // 2, 2, 128, KT, 128)
        .transpose(0, 1, 4, 2, 5, 3, 6)
    )
    rpc = np.ascontiguousarray(rp.reshape(NCORES, 1, BP * S))
    return [
        {"qst": qst[c], "pt": pt[c], "qs": qsn[c], "rp": rpc[c]}
        for c in range(NCORES)
    ]


def _postprocess(results):
    o = np.stack([np.asarray(r["out"], dtype=np.float32) for r in results])
    return np.ascontiguousarray(o.reshape(B, 1, S))


def _run(inputs, trace=False, **kw):
    nc = _get_nc()
    in_maps = _prep_inputs(inputs["p"], inputs["q"])
    res = run_bass_kernel_spmd(nc, in_maps, list(range(NCORES)), trace=trace, **kw)
    return _postprocess(res.results), res


def kernel(p, q):
    out, _ = _run({"p": p, "q": q})
    return out
